# revision 1
# baseline (speedup 1.0000x reference)
"""Weighted BCE loss (nn_BCELoss_with_weight) on 8 Trainium2 NeuronCores.

Reference computes:
    log_p   = max(log(pred), -100)            # clamp never binds: pred in [1e-4, 1-1e-4]
    log_1mp = max(log1p(-pred), -100)
    bce     = -(true*log_p + (1-true)*log_1mp)    # [B,C,D,H,W] = [2,16,64,128,128]
    per_class = mean(bce, axes=(0,2,3,4))         # [C]
    out = sum(weight*per_class) / sum(weight)     # scalar

Sharding: D=64 split into 8 slices of 8 (data parallel). Per core the shard
[2,16,8,128,128] is viewed as [B=2, (C,Dl)=128, H*W=16384]: partition p holds
class c=p//8 only, so the per-class weight is a per-partition scalar.

Per core on device, with u=ln(p), v=ln(1-p), w~=bf16(weight):
    term = t*u + (1-t)*v = t*(u-v) + v
    ACT : u = Ln(p) [bf16 out];  v = Ln(-p+1) [bf16 out, accum_out -> sum(v)]
    DVE : d = u - v (bf16 TT, 2x);  m = t*d (bf16 TT, 2x)   [t cast via SWDGE DMA]
    PE  : psum[1,512] += w~[128,1].T @ m[:,512-chunk]  (f32 accumulate)
    out_m[1,1]  = sum(psum)           -- already class-weighted
    out_v[128,1] = per-partition sum(v)
Host: result = -(sum_cores out_m + sum_p w~[p//8]*out_v[p]) / (M*sum(w~)),
with M = B*D*H*W.  Using the bf16-rounded weights consistently in both the
numerator and denominator makes this the exact weighted-mean of per-class BCE
with weights w~; since per-class means are ~equal, the w->w~ rounding
perturbs the result by ~|delta_w|*spread(per_class) ~ 1e-5 relative.
"""

import numpy as np

N_CORES = 8
B, C, D, H, W = 2, 16, 64, 128, 128
HW = H * W            # 16384 free elems per (b, partition)
P = 128               # (C=16) x (D_local=8) partitions
D_LOCAL = D // N_CORES
MM_N = 512            # one PSUM bank of f32


def _segments(free, n_b, taper, mid_chunk):
    """Per-b DMA segment sizes: tapered at stream start and end.

    taper: e.g. [2048, 2048, 4096] -> first b starts with the taper,
    last b ends with it reversed; the middle is filled with mid_chunk.
    """
    segs_per_b = []
    for b in range(n_b):
        head = list(taper) if b == 0 else []
        tail = list(reversed(taper)) if b == n_b - 1 else []
        mid_total = free - sum(head) - sum(tail)
        assert mid_total >= 0 and mid_total % mid_chunk == 0, (free, head, tail)
        segs_per_b.append(head + [mid_chunk] * (mid_total // mid_chunk) + tail)
    return segs_per_b


def build_bass_kernel(free=HW, n_b=B, sub=4096,
                      pin_bufs=3, tin_bufs=2, uv_bufs=3,
                      taper=(512, 512, 1024, 2048, 4096), mid_chunk=8192,
                      cast_t=True, split_p_rings=True):
    """Build the per-core Bass/Tile kernel.

    Inputs  : pred, true [n_b, 128, free] f32 (shard, class*d_local on axis 1)
              wf [128, 1] bf16 (per-partition class weight)
    Outputs : out_m [1, 1] f32   = sum_p wf[p] * sum_e (t*(u-v))[p, e]
              out_v [128, 1] f32 = per-partition sum_e v[p, e]
    """
    import concourse.bacc as bacc
    import concourse.mybir as mybir
    import concourse.tile as tile
    from concourse.alu_op_type import AluOpType

    f32 = mybir.dt.float32
    bf16 = mybir.dt.bfloat16
    AF = mybir.ActivationFunctionType

    segs_per_b = _segments(free, n_b, taper, mid_chunk)
    t_chunk = min(free, 8192)
    # flat list of (b, offset, seg_size, [sub-chunk sizes], new_t_chunk)
    plan = []
    ncols = 0
    total_mm = 0
    for b in range(n_b):
        off = 0
        for seg in segs_per_b[b]:
            subs = [sub] * (seg // sub) if seg >= sub else [seg]
            # t is cast in fixed big chunks (fewer SWDGE descriptor-gen
            # rounds on Q7); p segments must nest inside t chunks.
            assert (off % t_chunk) + seg <= t_chunk, (off, seg)
            plan.append((b, off, seg, subs, off % t_chunk == 0))
            ncols += len(subs)
            total_mm += seg // MM_N
            off += seg
        assert off == free

    nc = bacc.Bacc("TRN2", target_bir_lowering=False, debug=False,
                   num_devices=N_CORES)
    pred_d = nc.dram_tensor("pred", [n_b, P, free], f32, kind="ExternalInput")
    true_d = nc.dram_tensor("true", [n_b, P, free], f32, kind="ExternalInput")
    wf_d = nc.dram_tensor("wf", [P, 1], bf16, kind="ExternalInput")
    outm_d = nc.dram_tensor("out_m", [1, 1], f32, kind="ExternalOutput")
    outv_d = nc.dram_tensor("out_v", [P, 1], f32, kind="ExternalOutput")

    with tile.TileContext(nc) as tc:
        with (
            tc.tile_pool(name="pin", bufs=pin_bufs) as pin,
            tc.tile_pool(name="tin", bufs=tin_bufs) as tin,
            tc.tile_pool(name="uv", bufs=uv_bufs) as uvp,
            tc.tile_pool(name="small", bufs=1) as small,
            tc.tile_pool(name="psum", bufs=1, space="PSUM") as psump,
        ):
            bias0 = small.tile([P, 1], f32, tag="bias0")
            bias1 = small.tile([P, 1], f32, tag="bias1")
            nc.vector.memset(bias0[:], 0.0)
            nc.vector.memset(bias1[:], 1.0)
            wf_t = small.tile([P, 1], bf16, tag="wf")
            nc.sync.dma_start(wf_t[:], wf_d[:])
            vacc = small.tile([P, ncols], f32, tag="vacc")
            acc_m = psump.tile([1, MM_N], f32, tag="acc_m")
            # warm up the Ln table set at t~0 so the first real ACTIVATE
            # doesn't pay the ~2.7us ACT_TABLE_LOAD after its data lands
            warm = small.tile([P, 1], bf16, tag="warm")
            nc.scalar.activation(warm[:], bias1[:], AF.Ln,
                                 bias=bias0[:], scale=1.0)

            col = 0
            mm_i = 0
            t_t = None
            for pi, (b, off, seg, subs, new_t) in enumerate(plan):
                p_t = pin.tile([P, seg], f32, tag="p")
                sl = slice(off, off + seg)
                # alternate p across both HWDGE rings (SP + ACT) so the p
                # stream gets 2 of the 3 DMA rings in the SDMA round-robin
                p_eng = nc.scalar if (split_p_rings and pi % 2) else nc.sync
                p_eng.dma_start(p_t[:], pred_d[b, :, sl])
                if new_t:
                    tc_sz = min(t_chunk, free - off)
                    t_t = tin.tile([P, tc_sz], bf16 if cast_t else f32,
                                   tag="t")
                    tsl = slice(off, off + tc_sz)
                    if cast_t:
                        # f32 -> bf16 cast inline (SWDGE-only feature)
                        nc.gpsimd.dma_start(t_t[:], true_d[b, :, tsl])
                    else:
                        nc.sync.dma_start(t_t[:], true_d[b, :, tsl])
                s_off = 0
                for s_sz in subs:
                    ss = slice(s_off, s_off + s_sz)
                    t_off = (off % t_chunk) + s_off
                    tss = slice(t_off, t_off + s_sz)
                    u = uvp.tile([P, s_sz], bf16, tag="u")
                    v = uvp.tile([P, s_sz], bf16, tag="v")
                    # u = ln(p); v = ln(1 - p), vacc[:, col] = sum(v)
                    nc.scalar.activation(u[:], p_t[:, ss], AF.Ln,
                                         bias=bias0[:], scale=1.0)
                    nc.scalar.activation(v[:], p_t[:, ss], AF.Ln,
                                         bias=bias1[:], scale=-1.0,
                                         accum_out=vacc[:, col:col + 1])
                    # u <- d = u - v ; u <- m = t * d   (bf16 2x TT)
                    nc.vector.tensor_sub(u[:], u[:], v[:])
                    nc.vector.tensor_mul(u[:], t_t[:, tss], u[:])
                    # acc_m[1, 512] += wf.T @ m[:, 512-chunk]
                    for q in range(s_sz // MM_N):
                        nc.tensor.matmul(
                            acc_m[:],
                            wf_t[:],
                            u[:, q * MM_N:(q + 1) * MM_N],
                            start=(mm_i == 0),
                            stop=(mm_i == total_mm - 1),
                        )
                        mm_i += 1
                    s_off += s_sz
                    col += 1

            outv_t = small.tile([P, 1], f32, tag="outv")
            nc.vector.reduce_sum(outv_t[:], vacc[:], axis=mybir.AxisListType.X)
            nc.sync.dma_start(outv_d[:], outv_t[:])
            accm_sb = small.tile([1, MM_N], f32, tag="accm_sb")
            nc.vector.tensor_copy(accm_sb[:], acc_m[:])
            outm_t = small.tile([1, 1], f32, tag="outm")
            nc.vector.reduce_sum(outm_t[:], accm_sb[:], axis=mybir.AxisListType.X)
            nc.sync.dma_start(outm_d[:], outm_t[:])

    nc.compile()
    return nc


_NC_CACHE = {}


def _get_nc():
    if "nc" not in _NC_CACHE:
        import json
        import os

        opts = json.loads(os.environ.get("KERNEL_OPTS", "{}"))
        if "taper" in opts:
            opts["taper"] = tuple(opts["taper"])
        _NC_CACHE["nc"] = build_bass_kernel(**opts)
    return _NC_CACHE["nc"]


def _bf16_round(x):
    """Round f32 array to bf16 values (kept in f32 representation)."""
    xi = np.asarray(x, dtype=np.float32).view(np.uint32)
    rounded = ((xi + 0x7FFF + ((xi >> 16) & 1)) & 0xFFFF0000).astype(np.uint32)
    return rounded.view(np.float32)


def shard_inputs(pred, true, weight):
    """Full [B,C,D,H,W] -> per-core in_maps."""
    import ml_dtypes

    wtile = np.repeat(np.asarray(weight, np.float32), D_LOCAL).reshape(P, 1)
    wf = wtile.astype(ml_dtypes.bfloat16)
    in_maps = []
    for i in range(N_CORES):
        d0 = i * D_LOCAL
        ps = np.ascontiguousarray(
            pred[:, :, d0:d0 + D_LOCAL].reshape(B, P, HW))
        ts = np.ascontiguousarray(
            true[:, :, d0:d0 + D_LOCAL].reshape(B, P, HW))
        in_maps.append({"pred": ps, "true": ts, "wf": wf})
    return in_maps


def combine(out_ms, out_vs, weight):
    """out_ms [n_cores] scalars, out_vs [n_cores, 128]; weight [16] f32."""
    wt = _bf16_round(np.repeat(np.asarray(weight, np.float32), D_LOCAL))
    wt64 = wt.astype(np.float64)
    m = float(B * D * H * W)
    w_sum = wt64[::D_LOCAL].sum()          # sum of the 16 bf16 class weights
    total_v = (np.asarray(out_vs, np.float64).sum(axis=0) * wt64).sum()
    total_m = float(np.asarray(out_ms, np.float64).sum())
    return np.float32(-(total_m + total_v) / (m * w_sum))


def kernel(pred, true, weight, _trace=False):
    from concourse.bass_utils import run_bass_kernel_spmd

    nc = _get_nc()
    in_maps = shard_inputs(np.asarray(pred), np.asarray(true), weight)
    res = run_bass_kernel_spmd(nc, in_maps, core_ids=list(range(N_CORES)),
                               trace=_trace)
    out_ms = [r["out_m"][0, 0] for r in res.results]
    out_vs = [r["out_v"][:, 0] for r in res.results]
    out = combine(out_ms, out_vs, weight)
    if _trace:
        return out, res
    return out



# revision 7
# speedup vs baseline: 1.1264x; 1.1264x over previous
"""Weighted BCE loss (nn_BCELoss_with_weight) on 8 Trainium2 NeuronCores.

Reference computes:
    log_p   = max(log(pred), -100)
    log_1mp = max(log1p(-pred), -100)
    bce     = -(true*log_p + (1-true)*log_1mp)    # [B,C,D,H,W] = [2,16,64,128,128]
    per_class = mean(bce, axes=(0,2,3,4))         # [C]
    out = sum(weight*per_class) / sum(weight)     # scalar

Sharding: D=64 split into 8 slices of 8 (data parallel). Per core the shard
[2,16,8,128,128] is laid out as [(C,Dl)=128 partitions, B*H*W=32768 free]:
partition p holds class c=p//8 only. Both inputs are cast on host during
sharding (pred -> bf16 clamped to <= 1-2^-8 so Ln(1-p) stays finite,
true -> bf16), halving HBM traffic to 16 MiB/core; the bf16-pred curvature
bias in ln(1-p) is ~1.6e-3 relative (validated vs the f32 reference).

Per core on device, for each column chunk (u=ln(p), v=ln(1-p), t=true):
    ACT : u = Ln(p)            [bf16 out]
    ACT : v = Ln(-p+1)         [bf16 out, accum_out -> acc col   = sum(v)]
    DVE : ttr  u *= t, accum_out -> acc col = sum(t*u)
    DVE : ttr  v = -(t*v), accum_out -> acc col = sum(-t*v)
    (sum over acc cols) -> out[128, 1] = sum_e [t*u + (1-t)*v][p, e]
Host: result = -sum_p w[p//8]*out[p] / (M*sum(w)), M = B*D*H*W, all f64
with the exact f32 weights (no PE matmul, so no weight rounding).

ACT is the bottleneck engine: 2 Ln passes x 32768 cols = 65536 cycles
@1.2GHz = 54.6us busy per core; DMA (16 MiB @ ~358 GB/s = 47us) and DVE
(2 TTR passes = ~34us) hide underneath it.
"""

import numpy as np

N_CORES = 8
B, C, D, H, W = 2, 16, 64, 128, 128
HW = H * W
P = 128               # (C=16) x (D_local=8) partitions
D_LOCAL = D // N_CORES
FREE = B * HW         # 32768 free elems per partition

# largest bf16 < 1.0; pred is clamped here so Ln(1-p) >= Ln(2^-8)
BF16_MAX_LT1 = 1.0 - 2.0 ** -8


def _chunk_plan(free, head, tail, mid_chunk):
    """Column chunk sizes: tapered at stream start and end, mid filled."""
    head, tail = list(head), list(tail)
    mid_total = free - sum(head) - sum(tail)
    assert mid_total >= 0, (free, head, tail)
    n_mid = max(1, -(-mid_total // mid_chunk))
    base = mid_total // n_mid
    mids = [base + (1 if i < mid_total - base * n_mid else 0)
            for i in range(n_mid)]
    assert sum(head) + sum(mids) + sum(tail) == free
    return head + [m for m in mids if m] + tail


def build_bass_kernel(free=FREE,
                      head=(1024, 2048, 4096), tail=(2048, 1024),
                      mid_chunk=8192, p_bufs=3, t_bufs=2, uv_bufs=3,
                      split_rings=False, const_bias=True, warm=True):
    """Build the per-core Bass/Tile kernel.

    Inputs  : pred, true [128, free] bf16 (host-cast, class*d_local on axis 0)
    Outputs : out [128, 1] f32 = per-partition sum of t*ln(p) + (1-t)*ln(1-p)
    """
    import concourse.bacc as bacc
    import concourse.mybir as mybir
    import concourse.tile as tile
    from concourse.alu_op_type import AluOpType

    f32 = mybir.dt.float32
    bf16 = mybir.dt.bfloat16
    AF = mybir.ActivationFunctionType

    chunks = _chunk_plan(free, head, tail, mid_chunk)
    ncols = len(chunks)

    nc = bacc.Bacc("TRN2", target_bir_lowering=False, debug=False,
                   num_devices=N_CORES)
    pred_d = nc.dram_tensor("pred", [P, free], bf16, kind="ExternalInput")
    true_d = nc.dram_tensor("true", [P, free], bf16, kind="ExternalInput")
    out_d = nc.dram_tensor("out", [P, 1], f32, kind="ExternalOutput")

    with tile.TileContext(nc) as tc:
        with (
            tc.tile_pool(name="pin", bufs=p_bufs) as pin,
            tc.tile_pool(name="tin", bufs=t_bufs) as tin,
            tc.tile_pool(name="uv", bufs=uv_bufs) as uvp,
            tc.tile_pool(name="small", bufs=1) as small,
        ):
            acc = small.tile([P, 3 * ncols], f32, tag="acc")
            if const_bias:
                bias0, bias1 = 0.0, 1.0
            else:
                b0t = small.tile([P, 1], f32, tag="bias0")
                b1t = small.tile([P, 1], f32, tag="bias1")
                nc.vector.memset(b0t[:], 0.0)
                nc.vector.memset(b1t[:], 1.0)
                bias0, bias1 = b0t[:], b1t[:]
            # warm up the Ln table set so the first real ACTIVATE doesn't
            # pay the ~2.7us ACT_TABLE_LOAD after its data lands
            if warm:
                w0 = small.tile([P, 1], f32, tag="warm0")
                nc.vector.memset(w0[:], 1.0)
                wt = small.tile([P, 1], bf16, tag="warm")
                nc.scalar.activation(wt[:], w0[:], AF.Ln, bias=bias0, scale=1.0)

            off = 0
            for ci, seg in enumerate(chunks):
                sl = slice(off, off + seg)
                p_t = pin.tile([P, seg], bf16, tag="p")
                t_t = tin.tile([P, seg], bf16, tag="t")
                p_eng = nc.scalar if (split_rings and ci % 2) else nc.sync
                p_eng.dma_start(p_t[:], pred_d[:, sl])
                t_eng = nc.scalar if (split_rings and ci % 2 == 0) else nc.sync
                t_eng.dma_start(t_t[:], true_d[:, sl])
                u = uvp.tile([P, seg], bf16, tag="u")
                v = uvp.tile([P, seg], bf16, tag="v")
                # u = ln(p); v = ln(1 - p), acc col <- sum(v)
                nc.scalar.activation(u[:], p_t[:], AF.Ln, bias=bias0, scale=1.0)
                nc.scalar.activation(v[:], p_t[:], AF.Ln, bias=bias1, scale=-1.0,
                                     accum_out=acc[:, 3 * ci:3 * ci + 1])
                # u <- (t*1)*u,  acc col <- sum(t*u)
                # v <- (t*-1)*v, acc col <- sum(-t*v)
                nc.vector.scalar_tensor_tensor(
                    out=u[:], in0=t_t[:], scalar=1.0, in1=u[:],
                    op0=AluOpType.mult, op1=AluOpType.mult,
                    accum_out=acc[:, 3 * ci + 1:3 * ci + 2])
                nc.vector.scalar_tensor_tensor(
                    out=v[:], in0=t_t[:], scalar=-1.0, in1=v[:],
                    op0=AluOpType.mult, op1=AluOpType.mult,
                    accum_out=acc[:, 3 * ci + 2:3 * ci + 3])
                off += seg
            assert off == free

            out_t = small.tile([P, 1], f32, tag="out")
            nc.vector.reduce_sum(out_t[:], acc[:], axis=mybir.AxisListType.X)
            nc.sync.dma_start(out_d[:], out_t[:])

    nc.compile()
    return nc


_NC_CACHE = {}


def _get_nc():
    if "nc" not in _NC_CACHE:
        import json
        import os

        opts = json.loads(os.environ.get("KERNEL_OPTS", "{}"))
        for k in ("head", "tail"):
            if k in opts:
                opts[k] = tuple(opts[k])
        _NC_CACHE["nc"] = build_bass_kernel(**opts)
    return _NC_CACHE["nc"]


def shard_inputs(pred, true):
    """Full [B,C,D,H,W] f32 -> per-core in_maps of [128, 32768] bf16."""
    import ml_dtypes

    bf16 = ml_dtypes.bfloat16
    # [B,C,cores,Dl,HW] -> [cores, C, Dl, B, HW] -> [cores, 128, B*HW]
    pb = np.minimum(np.asarray(pred, np.float32).astype(bf16),
                    bf16(BF16_MAX_LT1))
    tb = np.asarray(true, np.float32).astype(bf16)
    def lay(x):
        x = x.reshape(B, C, N_CORES, D_LOCAL, HW)
        x = np.ascontiguousarray(x.transpose(2, 1, 3, 0, 4))
        return x.reshape(N_CORES, P, FREE)
    ps, ts = lay(pb), lay(tb)
    return [{"pred": ps[i], "true": ts[i]} for i in range(N_CORES)]


def combine(outs, weight):
    """outs [n_cores][128] f32 per-partition sums; weight [16] f32."""
    w64 = np.asarray(weight, np.float64)
    wp = np.repeat(w64, D_LOCAL)               # [128] per-partition weight
    m = float(B * D * H * W)
    total = (np.asarray(outs, np.float64).sum(axis=0) * wp).sum()
    return np.float32(-total / (m * w64.sum()))


def kernel(pred, true, weight, _trace=False):
    from concourse.bass_utils import run_bass_kernel_spmd

    nc = _get_nc()
    in_maps = shard_inputs(pred, true)
    res = run_bass_kernel_spmd(nc, in_maps, core_ids=list(range(N_CORES)),
                               trace=_trace)
    outs = [r["out"][:, 0] for r in res.results]
    out = combine(outs, weight)
    if _trace:
        return out, res
    return out


# revision 8
# speedup vs baseline: 1.4206x; 1.2612x over previous
"""Weighted BCE loss (nn_BCELoss_with_weight) on 8 Trainium2 NeuronCores.

Reference computes:
    log_p   = max(log(pred), -100)
    log_1mp = max(log1p(-pred), -100)
    bce     = -(true*log_p + (1-true)*log_1mp)    # [B,C,D,H,W] = [2,16,64,128,128]
    per_class = mean(bce, axes=(0,2,3,4))         # [C]
    out = sum(weight*per_class) / sum(weight)     # scalar

Sharding: D=64 split into 8 slices of 8 (data parallel). Per core the shard
[2,16,8,128,128] is laid out as [(C,Dl)=128 partitions, B*H*W=32768 free]:
partition p holds class c=p//8 only, so the class weight is a per-partition
scalar. Both inputs are cast on host during sharding (pred -> bf16 clamped
to <= 1-2^-8 so Ln(1-p) stays finite, true -> bf16), halving HBM traffic to
16 MiB/core; the bf16-pred curvature bias in ln(1-p) is ~1.6e-3 relative
(validated against the f32 reference; tolerance is 2e-2).

Per core on device, for each column chunk (u=ln(p), v=ln(1-p), t=true):
    ACT : u = Ln(p) [bf16 out];  v = Ln(-p+1) [bf16, accum_out -> sum(v)]
    DVE : u <- u - v (bf16 TT);  u <- t * u  (bf16 TT)
    PE  : psum[1,512] += wf[128,1].T @ (t*(u-v))[:, 512-chunk]  (f32 acc)
    out_m[1,1]  = sum(psum)          -- already class-weighted
    out_v[128,1] = per-partition sum(v)
Host: result = -(sum_cores out_m + sum_p wf[p]*out_v[p]) / (M*sum(wf)),
M = B*D*H*W, f64, with the bf16-rounded weights used consistently in
numerator and denominator (exact weighted mean w.r.t. the bf16 weights).

Engine budget per core (measured rates): ACT 2 Ln passes x 32768 cols @
~0.96ns/col ~ 67us (bottleneck); DVE 2 TT bf16 @ 0.52ns/col ~ 34us; PE 64
matmuls ~ 45us; DMA 16 MiB ~ 50us -- all hidden under ACT.
"""

import numpy as np

N_CORES = 8
B, C, D, H, W = 2, 16, 64, 128, 128
HW = H * W
P = 128               # (C=16) x (D_local=8) partitions
D_LOCAL = D // N_CORES
FREE = B * HW         # 32768 free elems per partition
MM_N = 512            # one PSUM bank of f32

# largest bf16 < 1.0; pred is clamped here so Ln(1-p) >= Ln(2^-8)
BF16_MAX_LT1 = 1.0 - 2.0 ** -8


def _chunk_plan(free, head, tail, mid_chunk):
    """Column chunk sizes: tapered at stream start and end, mid filled."""
    head, tail = list(head), list(tail)
    mid_total = free - sum(head) - sum(tail)
    assert mid_total >= 0, (free, head, tail)
    n_mid = max(1, -(-mid_total // mid_chunk))
    base = mid_total // n_mid
    rem = mid_total - base * n_mid
    mids = [base + (1 if i < rem else 0) for i in range(n_mid)]
    plan = head + [m for m in mids if m] + tail
    assert sum(plan) == free
    return plan


def build_bass_kernel(free=FREE,
                      head=(2048, 4096), tail=(2048,),
                      mid_chunk=8192, p_bufs=3, t_bufs=2, uv_bufs=3,
                      split_rings=False):
    """Build the per-core Bass/Tile kernel.

    Inputs  : pred, true [128, free] bf16 (host-cast)
              wf [128, 1] bf16 (per-partition class weight)
    Outputs : out_m [1, 1] f32   = sum_p wf[p] * sum_e (t*(u-v))[p, e]
              out_v [128, 1] f32 = per-partition sum_e v[p, e]
    """
    import concourse.bacc as bacc
    import concourse.mybir as mybir
    import concourse.tile as tile

    f32 = mybir.dt.float32
    bf16 = mybir.dt.bfloat16
    AF = mybir.ActivationFunctionType

    chunks = _chunk_plan(free, head, tail, mid_chunk)
    ncols = len(chunks)
    total_mm = sum(seg // MM_N for seg in chunks)

    nc = bacc.Bacc("TRN2", target_bir_lowering=False, debug=False,
                   num_devices=N_CORES)
    pred_d = nc.dram_tensor("pred", [P, free], bf16, kind="ExternalInput")
    true_d = nc.dram_tensor("true", [P, free], bf16, kind="ExternalInput")
    wf_d = nc.dram_tensor("wf", [P, 1], bf16, kind="ExternalInput")
    outm_d = nc.dram_tensor("out_m", [1, 1], f32, kind="ExternalOutput")
    outv_d = nc.dram_tensor("out_v", [P, 1], f32, kind="ExternalOutput")

    with tile.TileContext(nc) as tc:
        with (
            tc.tile_pool(name="pin", bufs=p_bufs) as pin,
            tc.tile_pool(name="tin", bufs=t_bufs) as tin,
            tc.tile_pool(name="uv", bufs=uv_bufs) as uvp,
            tc.tile_pool(name="small", bufs=1) as small,
            tc.tile_pool(name="psum", bufs=1, space="PSUM") as psump,
        ):
            wf_t = small.tile([P, 1], bf16, tag="wf")
            nc.sync.dma_start(wf_t[:], wf_d[:])
            vacc = small.tile([P, ncols], f32, tag="vacc")
            acc_m = psump.tile([1, MM_N], f32, tag="acc_m")
            # warm up the Ln table set so the first real ACTIVATE doesn't
            # pay the ACT_TABLE_LOAD after its data lands
            w0 = small.tile([P, 1], f32, tag="warm0")
            nc.vector.memset(w0[:], 1.0)
            wt = small.tile([P, 1], bf16, tag="warm")
            nc.scalar.activation(wt[:], w0[:], AF.Ln, bias=0.0, scale=1.0)

            off = 0
            mm_i = 0
            for ci, seg in enumerate(chunks):
                sl = slice(off, off + seg)
                p_t = pin.tile([P, seg], bf16, tag="p")
                t_t = tin.tile([P, seg], bf16, tag="t")
                p_eng = nc.scalar if (split_rings and ci % 2) else nc.sync
                p_eng.dma_start(p_t[:], pred_d[:, sl])
                t_eng = nc.scalar if (split_rings and ci % 2 == 0) else nc.sync
                t_eng.dma_start(t_t[:], true_d[:, sl])
                u = uvp.tile([P, seg], bf16, tag="u")
                v = uvp.tile([P, seg], bf16, tag="v")
                # u = ln(p); v = ln(1 - p), vacc col <- sum(v)
                nc.scalar.activation(u[:], p_t[:], AF.Ln, bias=0.0, scale=1.0)
                nc.scalar.activation(v[:], p_t[:], AF.Ln, bias=1.0, scale=-1.0,
                                     accum_out=vacc[:, ci:ci + 1])
                # u <- d = u - v ; u <- m = t * d   (bf16 TT)
                nc.vector.tensor_sub(u[:], u[:], v[:])
                nc.vector.tensor_mul(u[:], t_t[:], u[:])
                # acc_m[1, 512] += wf.T @ m[:, 512-chunk]
                for q in range(seg // MM_N):
                    nc.tensor.matmul(
                        acc_m[:],
                        wf_t[:],
                        u[:, q * MM_N:(q + 1) * MM_N],
                        start=(mm_i == 0),
                        stop=(mm_i == total_mm - 1),
                    )
                    mm_i += 1
                off += seg
            assert off == free and mm_i == total_mm

            outv_t = small.tile([P, 1], f32, tag="outv")
            nc.vector.reduce_sum(outv_t[:], vacc[:], axis=mybir.AxisListType.X)
            nc.sync.dma_start(outv_d[:], outv_t[:])
            accm_sb = small.tile([1, MM_N], f32, tag="accm_sb")
            nc.vector.tensor_copy(accm_sb[:], acc_m[:])
            outm_t = small.tile([1, 1], f32, tag="outm")
            nc.vector.reduce_sum(outm_t[:], accm_sb[:], axis=mybir.AxisListType.X)
            nc.sync.dma_start(outm_d[:], outm_t[:])

    nc.compile()
    return nc


_NC_CACHE = {}


def _get_nc():
    if "nc" not in _NC_CACHE:
        import json
        import os

        opts = json.loads(os.environ.get("KERNEL_OPTS", "{}"))
        for k in ("head", "tail"):
            if k in opts:
                opts[k] = tuple(opts[k])
        _NC_CACHE["nc"] = build_bass_kernel(**opts)
    return _NC_CACHE["nc"]


def _bf16_round(x):
    """Round f32 array to bf16 values (kept in f32 representation)."""
    xi = np.asarray(x, dtype=np.float32).view(np.uint32)
    rounded = ((xi + 0x7FFF + ((xi >> 16) & 1)) & 0xFFFF0000).astype(np.uint32)
    return rounded.view(np.float32)


def shard_inputs(pred, true, weight):
    """Full [B,C,D,H,W] f32 -> per-core in_maps of [128, 32768] bf16."""
    import ml_dtypes

    bf16 = ml_dtypes.bfloat16
    wtile = np.repeat(np.asarray(weight, np.float32), D_LOCAL).reshape(P, 1)
    wf = wtile.astype(bf16)
    pb = np.minimum(np.asarray(pred, np.float32).astype(bf16),
                    bf16(BF16_MAX_LT1))
    tb = np.asarray(true, np.float32).astype(bf16)

    def lay(x):
        # [B,C,cores,Dl,HW] -> [cores, C, Dl, B, HW] -> [cores, 128, B*HW]
        x = x.reshape(B, C, N_CORES, D_LOCAL, HW)
        x = np.ascontiguousarray(x.transpose(2, 1, 3, 0, 4))
        return x.reshape(N_CORES, P, FREE)

    ps, ts = lay(pb), lay(tb)
    return [{"pred": ps[i], "true": ts[i], "wf": wf}
            for i in range(N_CORES)]


def combine(out_ms, out_vs, weight):
    """out_ms [n_cores] scalars, out_vs [n_cores, 128]; weight [16] f32."""
    wt = _bf16_round(np.repeat(np.asarray(weight, np.float32), D_LOCAL))
    wt64 = wt.astype(np.float64)
    m = float(B * D * H * W)
    w_sum = wt64[::D_LOCAL].sum()          # sum of the 16 bf16 class weights
    total_v = (np.asarray(out_vs, np.float64).sum(axis=0) * wt64).sum()
    total_m = float(np.asarray(out_ms, np.float64).sum())
    return np.float32(-(total_m + total_v) / (m * w_sum))


def kernel(pred, true, weight, _trace=False):
    from concourse.bass_utils import run_bass_kernel_spmd

    nc = _get_nc()
    in_maps = shard_inputs(np.asarray(pred), np.asarray(true), weight)
    res = run_bass_kernel_spmd(nc, in_maps, core_ids=list(range(N_CORES)),
                               trace=_trace)
    out_ms = [r["out_m"][0, 0] for r in res.results]
    out_vs = [r["out_v"][:, 0] for r in res.results]
    out = combine(out_ms, out_vs, weight)
    if _trace:
        return out, res
    return out
